# revision 10
# baseline (speedup 1.0000x reference)
"""Fused MHA block (qkvg proj + biased masked softmax + sigmoid gating +
out proj + residual + LayerNorm) for one TRN2 chip — fp8 DoubleRow, v2.

Sharding: data parallel over batch. B=8 -> 8 NeuronCores, one batch element
per core, no collectives. Weights replicated.

Changes vs v1 (112.3us):
  * Bias+mask injection is FUSED into the scores matmul's second DoubleRow
    slot instead of a separate identity matmul: lhsT = [k-block | C*I] via a
    per-kt strided AP over a KI tile whose row 8 holds C*I; rhs = [q | BT_kt]
    via a per-kt strided AP over a QB tile whose rows 1..8 hold the
    TRANSPOSED bias (BT[p,kt,q] = (gamma*bias[q,kt*128+p]+OFF)*SU, masked ->
    -240).  Halves the scores PE cost (one DR matmul per (kt, 512q) chunk).
  * Scores PSUM is one [128,2,N] tile (4 banks); exp runs once per kt-PAIR
    over [128,2048], amortizing the ACT access-latency overhead (32 exps of
    2048 instead of 64 of 1024).
  * Gate path: t=exp(-g) (ACT) then ONE custom-DVE AFFINE_MUL_REDUCE
    den2 = (16t+16)*denom and r2 = 1/den2 (DVE), ATT = av*r2 (Pool).
    Removes the Pool affine + separate sigmoid reciprocal + one multiply.
  * rstd = exp(-0.5*ln(var+eps')): Ln+Exp live in the same ACT table
    (natural_log_exp_and_others) as the softmax Exp -> ONE table load total.
  * Residual rides the ff matmul as fp8 hi/lo DoubleRow (x16 = hi+lo, both
    slots through a stride-0 identity lhsT) instead of bf16 identity:
    halves residual PE cost, same precision (~0.4%).
  * LN normalize split ACT/Pool per 512-chunk to balance engines.
  * v-copies moved DVE -> Pool.
  * No zero-slot q/k tiles -> no big Pool memsets.

Scale ledger (fp8 ranges; e4m3 max finite = 240):
    Wq,Wk *8 ; Wv,Wg *64 ; x *1        -> q_ps=8q k_ps=8k v_ps=64v g_ps=64g
    exp arg = sc_ps * ES, ES=1/(64*sqrt(128)); bias via BT=(gamma*b+OFF)*SU,
        SU = 1/(ES*C), C=128, OFF=-3 (exp <= e^~2.5, fits fp8)
    sig path: t=exp(-g_ps/64)=e^-g (ACT); den2=(16t+16)*denom (DVE AMR);
        r2=1/den2 (DVE)
    ATT = av_ps * r2 = 4*attv*sig   (av_ps = 64*denom*attv)
    W_ff *4 -> ff_ps = 16*ff ; x16 = hi+lo fp8 ; h_ps = 16*(x+ff)
    eps' = 256*eps ; rstd = exp(-0.5*ln(var+eps'))
"""

import math

import numpy as np
import ml_dtypes

import concourse.bass as bass
import concourse.mybir as mybir
import concourse.tile as tile
from concourse import bacc
from concourse.bass_utils import run_bass_kernel_spmd

B, N, D, H, DH = 8, 1024, 1024, 8, 128
KT = D // 128
KTP = KT // 2
LN_EPS = 1e-5

F32 = mybir.dt.float32
BF16 = mybir.dt.bfloat16
FP8 = mybir.dt.float8e4
DR = mybir.MatmulPerfMode.DoubleRow
FP8NP = ml_dtypes.float8_e4m3
AF = mybir.ActivationFunctionType

SQ = 8.0        # q,k weight prescale
SV = 64.0       # v,g weight prescale
CID = 128.0     # identity-slot constant
ES = 1.0 / (SQ * SQ * math.sqrt(DH))     # exp() scale on scores psum
SU = 1.0 / (ES * CID)                    # bias prescale into BT
OFF = -3.0      # score offset (softmax-invariant), keeps exp in fp8 range
SA = 16.0       # h_ps = SA*(x+ff)
EPS2 = LN_EPS * SA * SA
NPAR = 4

_cache = {}


def _ident(nc, ap2d, fill):
    """diag(fill) into a zeroed [128,128] view."""
    nc.gpsimd.memset(ap2d, 0.0)
    nc.gpsimd.affine_select(
        out=ap2d,
        in_=ap2d,
        compare_op=mybir.AluOpType.not_equal,
        fill=fill,
        base=0,
        pattern=[[-1, 128]],
        channel_multiplier=1,
    )


def _build(flags):
    general_gamma, use_bff, use_lng, use_lnb = flags
    nc = bacc.Bacc("TRN2", target_bir_lowering=False)

    xt8_d = nc.dram_tensor("xt8", [128, KT, N], FP8, kind="ExternalInput")
    bt_shape = [H, 128, KT, N] if general_gamma else [128, KT, N]
    bt_d = nc.dram_tensor("bt", bt_shape, FP8, kind="ExternalInput")
    watt_d = nc.dram_tensor("watt", [H, 128, 4, KTP, 2, 128], FP8, kind="ExternalInput")
    wff_d = nc.dram_tensor("wff", [128, H, D], FP8, kind="ExternalInput")
    xhl_d = nc.dram_tensor("xhl", [KT, 128, 2, D], FP8, kind="ExternalInput")
    if use_bff:
        bff_d = nc.dram_tensor("bff", [1, D], F32, kind="ExternalInput")
    if use_lng:
        lng_d = nc.dram_tensor("lng", [1, D], F32, kind="ExternalInput")
    if use_lnb:
        lnb_d = nc.dram_tensor("lnb", [1, D], F32, kind="ExternalInput")
    out_d = nc.dram_tensor("out", [N, D], BF16, kind="ExternalOutput")

    with tile.TileContext(nc) as tc:
        with (
            tc.tile_pool(name="singles", bufs=1) as singles,
            tc.tile_pool(name="sb_w", bufs=3) as sb_w,
            tc.tile_pool(name="sb_sig", bufs=2) as sb_sig,
            tc.tile_pool(name="sb_v", bufs=3) as sb_v,
            tc.tile_pool(name="sb_dr", bufs=4) as sb_dr,
            tc.tile_pool(name="sb_st", bufs=4) as sb_st,
            tc.tile_pool(name="sb_o", bufs=3) as sb_o,
            tc.tile_pool(name="ps_sc", bufs=1, space="PSUM") as ps_sc,
            tc.tile_pool(name="ps_pr", bufs=2, space="PSUM") as ps_pr,
            tc.tile_pool(name="ps_tl", bufs=2, space="PSUM") as ps_tl,
        ):
            # ---- constants / resident tensors ----
            XT8 = singles.tile([128, KT, N], FP8, tag="XT8")
            wt0 = singles.tile([128, 4, KTP, 2, 128], FP8, tag="wt0")
            # head-0 critical path first: x^T half 0, then q/k/g weight
            # sections of head 0, then x^T half 1, then head-0 v weights.
            nc.sync.dma_start(out=XT8[:, :, 0:512], in_=xt8_d[:, :, 0:512])
            nc.sync.dma_start(out=wt0[:, 0:2, :, :, :], in_=watt_d[0, :, 0:2])
            nc.sync.dma_start(out=wt0[:, 3, :, :, :], in_=watt_d[0, :, 3])
            nc.sync.dma_start(out=XT8[:, :, 512:N], in_=xt8_d[:, :, 512:N])
            nc.sync.dma_start(out=wt0[:, 2, :, :, :], in_=watt_d[0, :, 2])

            QB = [singles.tile([128, 1 + KT, N], FP8, tag=f"QB{p}", name=f"QB{p}")
                  for p in range(NPAR)]
            KI = [singles.tile([128, KT + 1, 128], FP8, tag=f"KI{p}", name=f"KI{p}")
                  for p in range(NPAR)]
            PT2 = [singles.tile([128, KT, N], FP8, tag=f"PT{p}", name=f"PT{p}")
                   for p in range(NPAR)]
            if not general_gamma:
                # bias rows replicated into every QB buffer, staggered so
                # QB0 is ready first.
                for p in range(NPAR):
                    nc.sync.dma_start(
                        out=QB[p][:, 1 : 1 + KT, :], in_=bt_d[:, :, :]
                    )

            WFF8 = singles.tile([128, H, D], FP8, tag="WFF8")
            XR8 = singles.tile([128, KT, 2, D], FP8, tag="XR8")
            ATT = singles.tile([128, H, N], FP8, tag="ATT")
            ones2 = singles.tile([128, 2, 128], FP8, tag="ones2")
            nc.gpsimd.memset(ones2, 1.0)
            eps_t = singles.tile([128, 1], F32, tag="eps_t")
            nc.gpsimd.memset(eps_t, EPS2)
            ID1 = singles.tile([128, 128], FP8, tag="ID1")
            _ident(nc, ID1, 1.0)
            for p in range(NPAR):
                _ident(nc, KI[p][:, KT, :], CID)
            if use_bff:
                bffb = singles.tile([128, D], F32, tag="bffb")
                nc.sync.dma_start(
                    out=bffb,
                    in_=bass.AP(tensor=bff_d, offset=0, ap=[[0, 128], [1, D]]),
                )
            if use_lng:
                lngb = singles.tile([128, D], F32, tag="lngb")
                nc.sync.dma_start(
                    out=lngb,
                    in_=bass.AP(tensor=lng_d, offset=0, ap=[[0, 128], [1, D]]),
                )
            if use_lnb:
                lnbb = singles.tile([128, D], F32, tag="lnbb")
                nc.sync.dma_start(
                    out=lnbb,
                    in_=bass.AP(tensor=lnb_d, offset=0, ap=[[0, 128], [1, D]]),
                )

            # strided-AP helpers (slot-1 rides a different row of the tile)
            def ki_lhs(par, kt):
                t = KI[par][:, 0, :]
                return bass.AP(
                    tensor=t.tensor,
                    offset=kt * 128,
                    ap=[[(KT + 1) * 128, 128], [(KT - kt) * 128, 2], [1, 128]],
                )

            def qb_rhs(par, kt, c):
                t = QB[par][:, 0, :]
                return bass.AP(
                    tensor=t.tensor,
                    offset=c * 512,
                    ap=[[(1 + KT) * N, 128], [(1 + kt) * N, 2], [1, 512]],
                )

            def id_res():
                return bass.AP(
                    tensor=ID1[:, :].tensor,
                    offset=0,
                    ap=[[128, 128], [0, 2], [1, 128]],
                )

            # ---- per-head attention ----
            for h in range(H):
                par = h % NPAR
                QBp, KIp, PT = QB[par], KI[par], PT2[par]
                if h == 0:
                    wt = wt0
                else:
                    wt = sb_w.tile([128, 4, KTP, 2, 128], FP8, tag="wt", name="wt")
                    nc.sync.dma_start(out=wt, in_=watt_d[h])
                nc.sync.dma_start(out=XR8[:, h, :, :], in_=xhl_d[h])
                if h == 6:
                    nc.sync.dma_start(out=WFF8, in_=wff_d[:, :, :])
                if general_gamma:
                    nc.sync.dma_start(out=QBp[:, 1 : 1 + KT, :], in_=bt_d[h])

                # - q -> QB row 0, k -> KI rows 0..7 (fp8 copies via DVE) -
                for j, dst in ((0, None), (1, None)):
                    for c in range(2):
                        pr = ps_pr.tile([128, 512], F32, tag="ps_pr", name=f"pr{j}{c}")
                        for ktp in range(KTP):
                            nc.tensor.matmul(
                                pr,
                                wt[:, j, ktp, :, :],
                                XT8[:, 2 * ktp : 2 * ktp + 2, c * 512 : (c + 1) * 512],
                                start=(ktp == 0),
                                stop=(ktp == KTP - 1),
                                perf_mode=DR,
                            )
                        if j == 0:
                            o_ap = QBp[:, 0, c * 512 : (c + 1) * 512]
                        else:
                            o_ap = KIp[:, 4 * c : 4 * c + 4, :]
                        nc.vector.tensor_copy(out=o_ap, in_=pr)

                # - gate projection into SC row 0 -> t = exp(-g) (ACT),
                #   one [128,1024] instr; then t2 = 16t+16 (Pool, sbuf) -
                SC = ps_sc.tile([128, 2, N], F32, tag="ps_sc", name="sc")
                sig_t = sb_sig.tile([128, N], BF16, tag="sig_t", name="sig_t")
                for c in range(2):
                    gr = SC[:, 0, c * 512 : (c + 1) * 512]
                    for ktp in range(KTP):
                        nc.tensor.matmul(
                            gr,
                            wt[:, 3, ktp, :, :],
                            XT8[:, 2 * ktp : 2 * ktp + 2, c * 512 : (c + 1) * 512],
                            start=(ktp == 0),
                            stop=(ktp == KTP - 1),
                            perf_mode=DR,
                        )
                nc.scalar.activation(
                    out=sig_t,
                    in_=SC[:, 0, :],
                    func=AF.Exp,
                    scale=-1.0 / SV,
                )
                # t2 = 16t+16 (Pool, sbuf); sig_r = 1/t2 = sigmoid(g)/16 (DVE)
                sig_r = sb_sig.tile([128, N], BF16, tag="sig_r", name="sig_r")
                nc.gpsimd.tensor_scalar(
                    sig_r, sig_t, SA, SA,
                    mybir.AluOpType.mult, mybir.AluOpType.add,
                )
                with nc.allow_low_precision(reason="gate in bf16; |err|~0.4% ok"):
                    nc.vector.reciprocal(sig_r, sig_r)

                # - v projection (natural [k-token, dh] layout) -
                v8 = sb_v.tile([128, KT, 128], FP8, tag="v8", name="v8")

                # - scores: one fused (k.q + bias) DR matmul per (kt, 512q);
                #   exp per kt-PAIR over [128,2048]; v-proj interleaved -
                for kt in range(KT):
                    for c in range(2):
                        nc.tensor.matmul(
                            SC[:, kt % 2, c * 512 : (c + 1) * 512],
                            ki_lhs(par, kt),
                            qb_rhs(par, kt, c),
                            start=True,
                            stop=True,
                            perf_mode=DR,
                        )
                    if kt % 2 == 1:
                        nc.scalar.activation(
                            out=PT[:, kt - 1 : kt + 1, :],
                            in_=SC[:, :, :],
                            func=AF.Exp,
                            scale=ES,
                        )
                        if kt < 4:
                            # v chunk (kt//2) while ACT runs the exp
                            c = kt // 2
                            vr = ps_tl.tile(
                                [128, 4, 128], F32, tag="ps_tl", name=f"vr{c}"
                            )
                            for nb4 in range(4):
                                nb = 4 * c + nb4
                                for ktp in range(KTP):
                                    nc.tensor.matmul(
                                        vr[:, nb4, :],
                                        XT8[:, 2 * ktp : 2 * ktp + 2, nb * 128 : (nb + 1) * 128],
                                        wt[:, 2, ktp, :, :],
                                        start=(ktp == 0),
                                        stop=(ktp == KTP - 1),
                                        perf_mode=DR,
                                    )
                            nc.vector.tensor_copy(
                                out=v8[:, 4 * c : 4 * c + 4, :], in_=vr
                            )

                # - denominators -> rbb = 1/denom (DVE, pre-broadcast) -
                rbb = sb_dr.tile([128, N], F32, tag="rbb", name="rbb")
                for c in range(2):
                    dn = ps_tl.tile([128, 512], F32, tag="ps_tl", name=f"dn{c}")
                    for ktp in range(KTP):
                        nc.tensor.matmul(
                            dn,
                            ones2,
                            PT[:, 2 * ktp : 2 * ktp + 2, c * 512 : (c + 1) * 512],
                            start=(ktp == 0),
                            stop=(ktp == KTP - 1),
                            perf_mode=DR,
                        )
                    nc.vector.reciprocal(
                        rbb[:, c * 512 : (c + 1) * 512], dn
                    )

                # - av = v8^T @ PT ; t1 = av / t2 (DVE divide);
                #   ATT = t1 * rbb (Pool, sbuf-only) -
                for c in range(2):
                    av = ps_tl.tile([128, 512], F32, tag="ps_tl", name=f"av{c}")
                    for ktp in range(KTP):
                        nc.tensor.matmul(
                            av,
                            v8[:, 2 * ktp : 2 * ktp + 2, :],
                            PT[:, 2 * ktp : 2 * ktp + 2, c * 512 : (c + 1) * 512],
                            start=(ktp == 0),
                            stop=(ktp == KTP - 1),
                            perf_mode=DR,
                        )
                    t1 = sb_dr.tile([128, 512], BF16, tag="t1", name=f"t1{c}")
                    nc.vector.tensor_mul(
                        t1, av, sig_r[:, c * 512 : (c + 1) * 512]
                    )
                    nc.gpsimd.tensor_mul(
                        ATT[:, h, c * 512 : (c + 1) * 512],
                        t1,
                        rbb[:, c * 512 : (c + 1) * 512],
                    )

            # ---- output projection + residual + LayerNorm ----
            ffsc = None
            for nt in range(N // 128):
                ffs = []
                stats = sb_st.tile([128, 2, 6], F32, tag="stats", name="stats")
                r4 = nt % 4
                if r4 == 2:
                    ffsc = ps_sc.tile([128, 2, N], F32, tag="ps_sc", name="ffp")
                for c in range(2):
                    if r4 == 0:
                        ff = ps_pr.tile([128, 512], F32, tag="ps_pr", name=f"ff{c}")
                    elif r4 == 1:
                        ff = ps_tl.tile([128, 512], F32, tag="ps_tl", name=f"ff{c}")
                    else:
                        ff = ffsc[:, r4 - 2, c * 512 : (c + 1) * 512]
                    ffs.append(ff)
                    # residual: ff = I^T@xhi + I^T@xlo (fp8 hi/lo DoubleRow)
                    nc.tensor.matmul(
                        ff,
                        id_res(),
                        XR8[:, nt, :, c * 512 : (c + 1) * 512],
                        start=True,
                        stop=False,
                        perf_mode=DR,
                    )
                    for fp4 in range(KTP):
                        nc.tensor.matmul(
                            ff,
                            ATT[:, 2 * fp4 : 2 * fp4 + 2, nt * 128 : (nt + 1) * 128],
                            WFF8[:, 2 * fp4 : 2 * fp4 + 2, c * 512 : (c + 1) * 512],
                            start=False,
                            stop=(fp4 == KTP - 1),
                            perf_mode=DR,
                        )
                    if use_bff:
                        nc.vector.tensor_add(
                            ff, ff, bffb[:, c * 512 : (c + 1) * 512]
                        )
                    nc.vector.bn_stats(out=stats[:, c, :], in_=ff)
                mv = sb_st.tile([128, 2], F32, tag="mv", name="mv")
                nc.vector.bn_aggr(out=mv, in_=stats)
                # rstd = exp(-0.5*ln(var+eps')) — same ACT table as Exp
                lnv = sb_st.tile([128, 1], F32, tag="lnv", name="lnv")
                nc.scalar.activation(
                    out=lnv,
                    in_=mv[:, 1:2],
                    func=AF.Ln,
                    bias=eps_t,
                    scale=1.0,
                )
                rstd = sb_st.tile([128, 1], F32, tag="rstd", name="rstd")
                nc.scalar.activation(
                    out=rstd,
                    in_=lnv,
                    func=AF.Exp,
                    scale=-0.5,
                )
                mb = sb_st.tile([128, 1], F32, tag="mb", name="mb")
                nc.vector.tensor_scalar(
                    mb, mv[:, 0:1], -1.0, rstd,
                    mybir.AluOpType.mult, mybir.AluOpType.mult,
                )
                o = sb_o.tile([128, D], BF16, tag="o", name="o")
                for c in range(2):
                    if c == 0:
                        nc.scalar.activation(
                            out=o[:, 0:512],
                            in_=ffs[0],
                            func=AF.Identity,
                            bias=mb,
                            scale=rstd,
                        )
                    else:
                        nc.vector.tensor_scalar(
                            o[:, 512:D], ffs[1], rstd, mb,
                            mybir.AluOpType.mult, mybir.AluOpType.add,
                        )
                    if use_lng:
                        nc.vector.tensor_mul(
                            o[:, c * 512 : (c + 1) * 512],
                            o[:, c * 512 : (c + 1) * 512],
                            lngb[:, c * 512 : (c + 1) * 512],
                        )
                    if use_lnb:
                        nc.vector.tensor_add(
                            o[:, c * 512 : (c + 1) * 512],
                            o[:, c * 512 : (c + 1) * 512],
                            lnbb[:, c * 512 : (c + 1) * 512],
                        )
                    nc.sync.dma_start(
                        out=out_d[
                            nt * 128 : (nt + 1) * 128, c * 512 : (c + 1) * 512
                        ],
                        in_=o[:, c * 512 : (c + 1) * 512],
                    )

    nc.finalize()
    return nc


def get_nc(flags=(False, False, False, False)):
    if flags not in _cache:
        _cache[flags] = _build(flags)
    return _cache[flags]


def _fp8(a):
    return np.asarray(a, dtype=np.float32).astype(FP8NP)


def kernel(x, mask, bias, gamma_f, W_att, W_ff, b_ff, ln_g, ln_b):
    x = np.asarray(x, dtype=np.float32)
    mask = np.asarray(mask)
    bias = np.asarray(bias, dtype=np.float32)
    gamma_f = np.asarray(gamma_f, dtype=np.float32)
    W_att = np.asarray(W_att, dtype=np.float32)
    W_ff = np.asarray(W_ff, dtype=np.float32)
    b_ff = np.asarray(b_ff, dtype=np.float32)
    ln_g = np.asarray(ln_g, dtype=np.float32)
    ln_b = np.asarray(ln_b, dtype=np.float32)

    general_gamma = not np.all(gamma_f == 1.0)
    use_bff = bool(np.any(b_ff != 0.0))
    use_lng = not np.all(ln_g == 1.0)
    use_lnb = bool(np.any(ln_b != 0.0))
    flags = (general_gamma, use_bff, use_lng, use_lnb)
    nc = get_nc(flags)

    # watt8[h, p, j, ktp, i, fcol] = W_att[ktp*256 + i*128 + p, sect_j + h*128
    #   + fcol] * scale_j   (j: 0=q 1=k 2=v 3=g)
    w4 = W_att.reshape(KTP, 2, 128, 4, H, DH)  # [ktp, i, p, sect, h, fcol]
    watt8 = np.empty((H, 128, 4, KTP, 2, 128), dtype=FP8NP)
    scales = (SQ, SQ, SV, SV)
    for j in range(4):
        # -> [h, p, ktp, i, fcol]
        sect = np.transpose(w4[:, :, :, j, :, :], (3, 2, 0, 1, 4))
        watt8[:, :, j, :, :, :] = _fp8(sect * scales[j])

    wff8 = _fp8(4.0 * W_ff.reshape(H, 128, D).transpose(1, 0, 2))

    valid = ~mask[:, 0, :, :]  # [B, N, N] True where kept

    in_maps = []
    for b in range(B):
        # x^T tiled: XT8[p, kt, n] = x[n, kt*128 + p]
        xt8 = _fp8(np.ascontiguousarray(
            x[b].T.reshape(KT, 128, N).transpose(1, 0, 2)
        ))
        # BT[p, kt, q] = (gamma*bias[q, kt*128+p] + OFF)*SU, masked -> -240
        btr = np.ascontiguousarray(bias[b].T)  # [k, q]
        btr = btr.reshape(KT, 128, N).transpose(1, 0, 2)  # [p, kt, q]
        vtr = np.ascontiguousarray(valid[b].T).reshape(KT, 128, N).transpose(1, 0, 2)
        if general_gamma:
            bt8 = np.empty((H, 128, KT, N), dtype=FP8NP)
            for h in range(H):
                uh = np.clip((gamma_f[h] * btr + OFF) * SU, -239.0, 239.0)
                bt8[h] = np.where(vtr, uh, np.float32(-240.0)).astype(FP8NP)
        else:
            uh = np.clip((btr + OFF) * SU, -239.0, 239.0)
            bt8 = np.where(vtr, uh, np.float32(-240.0)).astype(FP8NP)
        # residual hi/lo: x16 = hi + lo in fp8
        x16 = SA * x[b]
        hi = x16.astype(FP8NP)
        lo = (x16 - hi.astype(np.float32)).astype(FP8NP)
        xhl = np.stack(
            [hi.reshape(KT, 128, D), lo.reshape(KT, 128, D)], axis=2
        )  # [KT, 128, 2, D]
        im = {
            "xt8": xt8,
            "bt": bt8,
            "watt": watt8,
            "wff": wff8,
            "xhl": np.ascontiguousarray(xhl),
        }
        if use_bff:
            im["bff"] = SA * b_ff.reshape(1, D)
        if use_lng:
            im["lng"] = ln_g.reshape(1, D)
        if use_lnb:
            im["lnb"] = ln_b.reshape(1, D)
        in_maps.append(im)

    res = run_bass_kernel_spmd(nc, in_maps, core_ids=list(range(B)))
    out = np.stack([res.results[b]["out"] for b in range(B)], axis=0)
    return out.astype(np.float32)


# revision 14
# speedup vs baseline: 1.3757x; 1.3757x over previous
"""Fused MHA block (qkvg proj + biased masked softmax + sigmoid gating +
out proj + residual + LayerNorm) for one TRN2 chip — fp8 DoubleRow, v2.

Sharding: data parallel over batch. B=8 -> 8 NeuronCores, one batch element
per core, no collectives. Weights replicated.

Changes vs v1 (112.3us):
  * Bias+mask injection is FUSED into the scores matmul's second DoubleRow
    slot instead of a separate identity matmul: lhsT = [k-block | C*I] via a
    per-kt strided AP over a KI tile whose row 8 holds C*I; rhs = [q | BT_kt]
    via a per-kt strided AP over a QB tile whose rows 1..8 hold the
    TRANSPOSED bias (BT[p,kt,q] = (gamma*bias[q,kt*128+p]+OFF)*SU, masked ->
    -240).  Halves the scores PE cost (one DR matmul per (kt, 512q) chunk).
  * Scores PSUM is one [128,2,N] tile (4 banks); exp runs once per kt-PAIR
    over [128,2048], amortizing the ACT access-latency overhead (32 exps of
    2048 instead of 64 of 1024).
  * Gate path: t=exp(-g) (ACT) then ONE custom-DVE AFFINE_MUL_REDUCE
    den2 = (16t+16)*denom and r2 = 1/den2 (DVE), ATT = av*r2 (Pool).
    Removes the Pool affine + separate sigmoid reciprocal + one multiply.
  * rstd = exp(-0.5*ln(var+eps')): Ln+Exp live in the same ACT table
    (natural_log_exp_and_others) as the softmax Exp -> ONE table load total.
  * Residual rides the ff matmul as fp8 hi/lo DoubleRow (x16 = hi+lo, both
    slots through a stride-0 identity lhsT) instead of bf16 identity:
    halves residual PE cost, same precision (~0.4%).
  * LN normalize split ACT/Pool per 512-chunk to balance engines.
  * v-copies moved DVE -> Pool.
  * No zero-slot q/k tiles -> no big Pool memsets.

Scale ledger (fp8 ranges; e4m3 max finite = 240):
    Wq,Wk *8 ; Wv,Wg *64 ; x *1        -> q_ps=8q k_ps=8k v_ps=64v g_ps=64g
    exp arg = sc_ps * ES, ES=1/(64*sqrt(128)); bias via BT=(gamma*b+OFF)*SU,
        SU = 1/(ES*C), C=128, OFF=-3 (exp <= e^~2.5, fits fp8)
    sig path: t=exp(-g_ps/64)=e^-g (ACT); den2=(16t+16)*denom (DVE AMR);
        r2=1/den2 (DVE)
    ATT = av_ps * r2 = 4*attv*sig   (av_ps = 64*denom*attv)
    W_ff *4 -> ff_ps = 16*ff ; x16 = hi+lo fp8 ; h_ps = 16*(x+ff)
    eps' = 256*eps ; rstd = exp(-0.5*ln(var+eps'))
"""

import math

import numpy as np
import ml_dtypes

import concourse.bass as bass
import concourse.mybir as mybir
import concourse.tile as tile
from concourse import bacc
from concourse.bass_utils import run_bass_kernel_spmd

B, N, D, H, DH = 8, 1024, 1024, 8, 128
KT = D // 128
KTP = KT // 2
LN_EPS = 1e-5

F32 = mybir.dt.float32
BF16 = mybir.dt.bfloat16
FP8 = mybir.dt.float8e4
DR = mybir.MatmulPerfMode.DoubleRow
FP8NP = ml_dtypes.float8_e4m3
AF = mybir.ActivationFunctionType

SQ = 8.0        # q,k weight prescale
SV = 64.0       # v,g weight prescale
CID = 128.0     # identity-slot constant
ES = 1.0 / (SQ * SQ * math.sqrt(DH))     # exp() scale on scores psum
SU = 1.0 / (ES * CID)                    # bias prescale into BT
OFF = -3.0      # score offset (softmax-invariant), keeps exp in fp8 range
SA = 16.0       # h_ps = SA*(x+ff)
EPS2 = LN_EPS * SA * SA
NPAR = 4

_cache = {}


def _ident(nc, ap2d, fill):
    """diag(fill) into a zeroed [128,128] view."""
    nc.gpsimd.memset(ap2d, 0.0)
    nc.gpsimd.affine_select(
        out=ap2d,
        in_=ap2d,
        compare_op=mybir.AluOpType.not_equal,
        fill=fill,
        base=0,
        pattern=[[-1, 128]],
        channel_multiplier=1,
    )


def _build(flags):
    general_gamma, use_bff, use_lng, use_lnb = flags
    nc = bacc.Bacc("TRN2", target_bir_lowering=False)

    xt8_d = nc.dram_tensor("xt8", [128, KT, N], FP8, kind="ExternalInput")
    bt_shape = [H, 128, KT, N] if general_gamma else [128, KT, N]
    bt_d = nc.dram_tensor("bt", bt_shape, FP8, kind="ExternalInput")
    watt_d = nc.dram_tensor("watt", [H, 128, 4, KTP, 2, 128], FP8, kind="ExternalInput")
    wff_d = nc.dram_tensor("wff", [128, H, D], FP8, kind="ExternalInput")
    xhl_d = nc.dram_tensor("xhl", [KT, 128, 2, D], FP8, kind="ExternalInput")
    if use_bff:
        bff_d = nc.dram_tensor("bff", [1, D], F32, kind="ExternalInput")
    if use_lng:
        lng_d = nc.dram_tensor("lng", [1, D], F32, kind="ExternalInput")
    if use_lnb:
        lnb_d = nc.dram_tensor("lnb", [1, D], F32, kind="ExternalInput")
    out_d = nc.dram_tensor("out", [N, D], BF16, kind="ExternalOutput")

    with tile.TileContext(nc) as tc:
        with (
            tc.tile_pool(name="singles", bufs=1) as singles,
            tc.tile_pool(name="sb_w", bufs=3) as sb_w,
            tc.tile_pool(name="sb_sig", bufs=2) as sb_sig,
            tc.tile_pool(name="sb_v", bufs=3) as sb_v,
            tc.tile_pool(name="sb_dr", bufs=4) as sb_dr,
            tc.tile_pool(name="sb_st", bufs=4) as sb_st,
            tc.tile_pool(name="sb_o", bufs=3) as sb_o,
            tc.tile_pool(name="ps_sc", bufs=2, space="PSUM") as ps_sc,
            tc.tile_pool(name="ps_pr", bufs=2, space="PSUM") as ps_pr,
            tc.tile_pool(name="ps_tl", bufs=2, space="PSUM") as ps_tl,
        ):
            # ---- constants / resident tensors ----
            XT8 = singles.tile([128, KT, N], FP8, tag="XT8")
            wt0 = singles.tile([128, 4, KTP, 2, 128], FP8, tag="wt0")
            # head-0 critical path first: x^T half 0, then q/k/g weight
            # sections of head 0, then x^T half 1, then head-0 v weights.
            nc.sync.dma_start(out=XT8[:, :, 0:512], in_=xt8_d[:, :, 0:512])
            nc.sync.dma_start(out=wt0[:, 0:2, :, :, :], in_=watt_d[0, :, 0:2])
            nc.sync.dma_start(out=wt0[:, 3, :, :, :], in_=watt_d[0, :, 3])
            nc.sync.dma_start(out=XT8[:, :, 512:N], in_=xt8_d[:, :, 512:N])
            nc.sync.dma_start(out=wt0[:, 2, :, :, :], in_=watt_d[0, :, 2])

            QB = [singles.tile([128, 1 + KT, N], FP8, tag=f"QB{p}", name=f"QB{p}")
                  for p in range(NPAR)]
            KI = [singles.tile([128, KT + 1, 128], FP8, tag=f"KI{p}", name=f"KI{p}")
                  for p in range(NPAR)]
            PT2 = [singles.tile([128, KT, N], FP8, tag=f"PT{p}", name=f"PT{p}")
                   for p in range(NPAR)]
            if not general_gamma:
                # bias rows replicated into every QB buffer, staggered so
                # QB0 is ready first.
                for p in range(NPAR):
                    nc.sync.dma_start(
                        out=QB[p][:, 1 : 1 + KT, :], in_=bt_d[:, :, :]
                    )

            WFF8 = singles.tile([128, H, D], FP8, tag="WFF8")
            XR8 = singles.tile([128, KT, 2, D], FP8, tag="XR8")
            ATT = singles.tile([128, H, N], FP8, tag="ATT")
            ones2 = singles.tile([128, 2, 128], FP8, tag="ones2")
            nc.gpsimd.memset(ones2, 1.0)
            eps_t = singles.tile([128, 1], F32, tag="eps_t")
            nc.gpsimd.memset(eps_t, EPS2)
            ID1 = singles.tile([128, 128], FP8, tag="ID1")
            _ident(nc, ID1, 1.0)
            for p in range(NPAR):
                _ident(nc, KI[p][:, KT, :], CID)
            if use_bff:
                bffb = singles.tile([128, D], F32, tag="bffb")
                nc.sync.dma_start(
                    out=bffb,
                    in_=bass.AP(tensor=bff_d, offset=0, ap=[[0, 128], [1, D]]),
                )
            if use_lng:
                lngb = singles.tile([128, D], F32, tag="lngb")
                nc.sync.dma_start(
                    out=lngb,
                    in_=bass.AP(tensor=lng_d, offset=0, ap=[[0, 128], [1, D]]),
                )
            if use_lnb:
                lnbb = singles.tile([128, D], F32, tag="lnbb")
                nc.sync.dma_start(
                    out=lnbb,
                    in_=bass.AP(tensor=lnb_d, offset=0, ap=[[0, 128], [1, D]]),
                )

            # strided-AP helpers (slot-1 rides a different row of the tile)
            def ki_lhs(par, kt):
                t = KI[par][:, 0, :]
                return bass.AP(
                    tensor=t.tensor,
                    offset=kt * 128,
                    ap=[[(KT + 1) * 128, 128], [(KT - kt) * 128, 2], [1, 128]],
                )

            def qb_rhs(par, kt, c):
                t = QB[par][:, 0, :]
                return bass.AP(
                    tensor=t.tensor,
                    offset=c * 512,
                    ap=[[(1 + KT) * N, 128], [(1 + kt) * N, 2], [1, 512]],
                )

            def id_res():
                return bass.AP(
                    tensor=ID1[:, :].tensor,
                    offset=0,
                    ap=[[128, 128], [0, 2], [1, 128]],
                )

            # ---- per-head attention ----
            for h in range(H):
                par = h % NPAR
                QBp, KIp, PT = QB[par], KI[par], PT2[par]
                if h == 0:
                    wt = wt0
                else:
                    wt = sb_w.tile([128, 4, KTP, 2, 128], FP8, tag="wt", name="wt")
                    nc.sync.dma_start(out=wt, in_=watt_d[h])
                nc.sync.dma_start(out=XR8[:, h, :, :], in_=xhl_d[h])
                if h == 6:
                    nc.sync.dma_start(out=WFF8, in_=wff_d[:, :, :])
                if general_gamma:
                    nc.sync.dma_start(out=QBp[:, 1 : 1 + KT, :], in_=bt_d[h])

                # - q -> QB row 0, k -> KI rows 0..7 (fp8 copies via DVE) -
                for j, dst in ((0, None), (1, None)):
                    for c in range(2):
                        pr = ps_pr.tile([128, 512], F32, tag="ps_pr", name=f"pr{j}{c}")
                        for ktp in range(KTP):
                            nc.tensor.matmul(
                                pr,
                                wt[:, j, ktp, :, :],
                                XT8[:, 2 * ktp : 2 * ktp + 2, c * 512 : (c + 1) * 512],
                                start=(ktp == 0),
                                stop=(ktp == KTP - 1),
                                perf_mode=DR,
                            )
                        if j == 0:
                            o_ap = QBp[:, 0, c * 512 : (c + 1) * 512]
                        else:
                            o_ap = KIp[:, 4 * c : 4 * c + 4, :]
                        nc.vector.tensor_copy(out=o_ap, in_=pr)

                # - gate projection into one scores-psum tile -> ONE
                #   t = exp(-g) over [128,1024] (ACT) -
                gp = ps_sc.tile([128, N], F32, tag="ps_sc", name="gp")
                sig_t = sb_sig.tile([128, N], BF16, tag="sig_t", name="sig_t")
                for c in range(2):
                    gr = gp[:, c * 512 : (c + 1) * 512]
                    for ktp in range(KTP):
                        nc.tensor.matmul(
                            gr,
                            wt[:, 3, ktp, :, :],
                            XT8[:, 2 * ktp : 2 * ktp + 2, c * 512 : (c + 1) * 512],
                            start=(ktp == 0),
                            stop=(ktp == KTP - 1),
                            perf_mode=DR,
                        )
                nc.scalar.activation(
                    out=sig_t,
                    in_=gp,
                    func=AF.Exp,
                    scale=-1.0 / SV,
                )
                # t2 = 16t+16 (Pool, sbuf); sig_r = 1/t2 = sigmoid(g)/16 (DVE)
                sig_r = sb_sig.tile([128, N], BF16, tag="sig_r", name="sig_r")
                nc.gpsimd.tensor_scalar(
                    sig_r, sig_t, SA, SA,
                    mybir.AluOpType.mult, mybir.AluOpType.add,
                )
                with nc.allow_low_precision(reason="gate in bf16; |err|~0.4% ok"):
                    nc.vector.reciprocal(sig_r, sig_r)

                # - v projection (natural [k-token, dh] layout) -
                v8 = sb_v.tile([128, KT, 128], FP8, tag="v8", name="v8")

                # - scores: one fused (k.q + bias) DR matmul per (kt, 512q);
                #   per-kt exp on ping-ponged [128,N] tiles; v interleaved -
                for kt in range(KT):
                    sc = ps_sc.tile([128, N], F32, tag="ps_sc", name=f"sc{kt}")
                    for c in range(2):
                        nc.tensor.matmul(
                            sc[:, c * 512 : (c + 1) * 512],
                            ki_lhs(par, kt),
                            qb_rhs(par, kt, c),
                            start=True,
                            stop=True,
                            perf_mode=DR,
                        )
                    nc.scalar.activation(
                        out=PT[:, kt, :],
                        in_=sc,
                        func=AF.Exp,
                        scale=ES,
                    )
                    if kt == 1 or kt == 3:
                        # v chunk while ACT chews on the exps
                        c = (kt - 1) // 2
                        vr = ps_tl.tile(
                            [128, 4, 128], F32, tag="ps_tl", name=f"vr{c}"
                        )
                        for nb4 in range(4):
                            nb = 4 * c + nb4
                            for ktp in range(KTP):
                                nc.tensor.matmul(
                                    vr[:, nb4, :],
                                    XT8[:, 2 * ktp : 2 * ktp + 2, nb * 128 : (nb + 1) * 128],
                                    wt[:, 2, ktp, :, :],
                                    start=(ktp == 0),
                                    stop=(ktp == KTP - 1),
                                    perf_mode=DR,
                                )
                        nc.vector.tensor_copy(
                            out=v8[:, 4 * c : 4 * c + 4, :], in_=vr
                        )

                # - denominators -> rbb = 1/denom (DVE, pre-broadcast) -
                rbb = sb_dr.tile([128, N], F32, tag="rbb", name="rbb")
                for c in range(2):
                    dn = ps_tl.tile([128, 512], F32, tag="ps_tl", name=f"dn{c}")
                    for ktp in range(KTP):
                        nc.tensor.matmul(
                            dn,
                            ones2,
                            PT[:, 2 * ktp : 2 * ktp + 2, c * 512 : (c + 1) * 512],
                            start=(ktp == 0),
                            stop=(ktp == KTP - 1),
                            perf_mode=DR,
                        )
                    nc.vector.reciprocal(
                        rbb[:, c * 512 : (c + 1) * 512], dn
                    )

                # - av = v8^T @ PT ; t1 = av / t2 (DVE divide);
                #   ATT = t1 * rbb (Pool, sbuf-only) -
                for c in range(2):
                    av = ps_tl.tile([128, 512], F32, tag="ps_tl", name=f"av{c}")
                    for ktp in range(KTP):
                        nc.tensor.matmul(
                            av,
                            v8[:, 2 * ktp : 2 * ktp + 2, :],
                            PT[:, 2 * ktp : 2 * ktp + 2, c * 512 : (c + 1) * 512],
                            start=(ktp == 0),
                            stop=(ktp == KTP - 1),
                            perf_mode=DR,
                        )
                    t1 = sb_dr.tile([128, 512], BF16, tag="t1", name=f"t1{c}")
                    nc.vector.tensor_mul(
                        t1, av, sig_r[:, c * 512 : (c + 1) * 512]
                    )
                    nc.gpsimd.tensor_mul(
                        ATT[:, h, c * 512 : (c + 1) * 512],
                        t1,
                        rbb[:, c * 512 : (c + 1) * 512],
                    )

            # ---- output projection + residual + LayerNorm ----
            for nt in range(N // 128):
                ffs = []
                stats = sb_st.tile([128, 2, 6], F32, tag="stats", name="stats")
                r4 = nt % 4
                if r4 >= 2:
                    ffsc = ps_sc.tile([128, N], F32, tag="ps_sc", name="ffp")
                for c in range(2):
                    if r4 == 0:
                        ff = ps_pr.tile([128, 512], F32, tag="ps_pr", name=f"ff{c}")
                    elif r4 == 1:
                        ff = ps_tl.tile([128, 512], F32, tag="ps_tl", name=f"ff{c}")
                    else:
                        ff = ffsc[:, c * 512 : (c + 1) * 512]
                    ffs.append(ff)
                    # residual: ff = I^T@xhi + I^T@xlo (fp8 hi/lo DoubleRow)
                    nc.tensor.matmul(
                        ff,
                        id_res(),
                        XR8[:, nt, :, c * 512 : (c + 1) * 512],
                        start=True,
                        stop=False,
                        perf_mode=DR,
                    )
                    for fp4 in range(KTP):
                        nc.tensor.matmul(
                            ff,
                            ATT[:, 2 * fp4 : 2 * fp4 + 2, nt * 128 : (nt + 1) * 128],
                            WFF8[:, 2 * fp4 : 2 * fp4 + 2, c * 512 : (c + 1) * 512],
                            start=False,
                            stop=(fp4 == KTP - 1),
                            perf_mode=DR,
                        )
                    if use_bff:
                        nc.vector.tensor_add(
                            ff, ff, bffb[:, c * 512 : (c + 1) * 512]
                        )
                    nc.vector.bn_stats(out=stats[:, c, :], in_=ff)
                mv = sb_st.tile([128, 2], F32, tag="mv", name="mv")
                nc.vector.bn_aggr(out=mv, in_=stats)
                rstd = sb_st.tile([128, 1], F32, tag="rstd", name="rstd")
                nc.scalar.activation(
                    out=rstd,
                    in_=mv[:, 1:2],
                    func=AF.Abs_reciprocal_sqrt,
                    bias=eps_t,
                    scale=1.0,
                )
                mb = sb_st.tile([128, 1], F32, tag="mb", name="mb")
                nc.vector.tensor_scalar(
                    mb, mv[:, 0:1], -1.0, rstd,
                    mybir.AluOpType.mult, mybir.AluOpType.mult,
                )
                o = sb_o.tile([128, D], BF16, tag="o", name="o")
                for c in range(2):
                    if c == 0:
                        nc.scalar.activation(
                            out=o[:, 0:512],
                            in_=ffs[0],
                            func=AF.Identity,
                            bias=mb,
                            scale=rstd,
                        )
                    else:
                        nc.vector.tensor_scalar(
                            o[:, 512:D], ffs[1], rstd, mb,
                            mybir.AluOpType.mult, mybir.AluOpType.add,
                        )
                    if use_lng:
                        nc.vector.tensor_mul(
                            o[:, c * 512 : (c + 1) * 512],
                            o[:, c * 512 : (c + 1) * 512],
                            lngb[:, c * 512 : (c + 1) * 512],
                        )
                    if use_lnb:
                        nc.vector.tensor_add(
                            o[:, c * 512 : (c + 1) * 512],
                            o[:, c * 512 : (c + 1) * 512],
                            lnbb[:, c * 512 : (c + 1) * 512],
                        )
                    nc.sync.dma_start(
                        out=out_d[
                            nt * 128 : (nt + 1) * 128, c * 512 : (c + 1) * 512
                        ],
                        in_=o[:, c * 512 : (c + 1) * 512],
                    )

    nc.finalize()
    return nc


def get_nc(flags=(False, False, False, False)):
    if flags not in _cache:
        _cache[flags] = _build(flags)
    return _cache[flags]


def _fp8(a):
    return np.asarray(a, dtype=np.float32).astype(FP8NP)


def kernel(x, mask, bias, gamma_f, W_att, W_ff, b_ff, ln_g, ln_b):
    x = np.asarray(x, dtype=np.float32)
    mask = np.asarray(mask)
    bias = np.asarray(bias, dtype=np.float32)
    gamma_f = np.asarray(gamma_f, dtype=np.float32)
    W_att = np.asarray(W_att, dtype=np.float32)
    W_ff = np.asarray(W_ff, dtype=np.float32)
    b_ff = np.asarray(b_ff, dtype=np.float32)
    ln_g = np.asarray(ln_g, dtype=np.float32)
    ln_b = np.asarray(ln_b, dtype=np.float32)

    general_gamma = not np.all(gamma_f == 1.0)
    use_bff = bool(np.any(b_ff != 0.0))
    use_lng = not np.all(ln_g == 1.0)
    use_lnb = bool(np.any(ln_b != 0.0))
    flags = (general_gamma, use_bff, use_lng, use_lnb)
    nc = get_nc(flags)

    # watt8[h, p, j, ktp, i, fcol] = W_att[ktp*256 + i*128 + p, sect_j + h*128
    #   + fcol] * scale_j   (j: 0=q 1=k 2=v 3=g)
    w4 = W_att.reshape(KTP, 2, 128, 4, H, DH)  # [ktp, i, p, sect, h, fcol]
    watt8 = np.empty((H, 128, 4, KTP, 2, 128), dtype=FP8NP)
    scales = (SQ, SQ, SV, SV)
    for j in range(4):
        # -> [h, p, ktp, i, fcol]
        sect = np.transpose(w4[:, :, :, j, :, :], (3, 2, 0, 1, 4))
        watt8[:, :, j, :, :, :] = _fp8(sect * scales[j])

    wff8 = _fp8(4.0 * W_ff.reshape(H, 128, D).transpose(1, 0, 2))

    valid = ~mask[:, 0, :, :]  # [B, N, N] True where kept

    in_maps = []
    for b in range(B):
        # x^T tiled: XT8[p, kt, n] = x[n, kt*128 + p]
        xt8 = _fp8(np.ascontiguousarray(
            x[b].T.reshape(KT, 128, N).transpose(1, 0, 2)
        ))
        # BT[p, kt, q] = (gamma*bias[q, kt*128+p] + OFF)*SU, masked -> -240
        btr = np.ascontiguousarray(bias[b].T)  # [k, q]
        btr = btr.reshape(KT, 128, N).transpose(1, 0, 2)  # [p, kt, q]
        vtr = np.ascontiguousarray(valid[b].T).reshape(KT, 128, N).transpose(1, 0, 2)
        if general_gamma:
            bt8 = np.empty((H, 128, KT, N), dtype=FP8NP)
            for h in range(H):
                uh = np.clip((gamma_f[h] * btr + OFF) * SU, -239.0, 239.0)
                bt8[h] = np.where(vtr, uh, np.float32(-240.0)).astype(FP8NP)
        else:
            uh = np.clip((btr + OFF) * SU, -239.0, 239.0)
            bt8 = np.where(vtr, uh, np.float32(-240.0)).astype(FP8NP)
        # residual hi/lo: x16 = hi + lo in fp8
        x16 = SA * x[b]
        hi = x16.astype(FP8NP)
        lo = (x16 - hi.astype(np.float32)).astype(FP8NP)
        xhl = np.stack(
            [hi.reshape(KT, 128, D), lo.reshape(KT, 128, D)], axis=2
        )  # [KT, 128, 2, D]
        im = {
            "xt8": xt8,
            "bt": bt8,
            "watt": watt8,
            "wff": wff8,
            "xhl": np.ascontiguousarray(xhl),
        }
        if use_bff:
            im["bff"] = SA * b_ff.reshape(1, D)
        if use_lng:
            im["lng"] = ln_g.reshape(1, D)
        if use_lnb:
            im["lnb"] = ln_b.reshape(1, D)
        in_maps.append(im)

    res = run_bass_kernel_spmd(nc, in_maps, core_ids=list(range(B)))
    out = np.stack([res.results[b]["out"] for b in range(B)], axis=0)
    return out.astype(np.float32)


# revision 18
# speedup vs baseline: 1.4108x; 1.0255x over previous
"""Fused MHA block (qkvg proj + biased masked softmax + sigmoid gating +
out proj + residual + LayerNorm) for one TRN2 chip — fp8 DoubleRow, v2.

Sharding: data parallel over batch. B=8 -> 8 NeuronCores, one batch element
per core, no collectives. Weights replicated.

Changes vs v1 (112.3us):
  * Bias+mask injection is FUSED into the scores matmul's second DoubleRow
    slot instead of a separate identity matmul: lhsT = [k-block | C*I] via a
    per-kt strided AP over a KI tile whose row 8 holds C*I; rhs = [q | BT_kt]
    via a per-kt strided AP over a QB tile whose rows 1..8 hold the
    TRANSPOSED bias (BT[p,kt,q] = (gamma*bias[q,kt*128+p]+OFF)*SU, masked ->
    -240).  Halves the scores PE cost (one DR matmul per (kt, 512q) chunk).
  * Scores PSUM is one [128,2,N] tile (4 banks); exp runs once per kt-PAIR
    over [128,2048], amortizing the ACT access-latency overhead (32 exps of
    2048 instead of 64 of 1024).
  * Gate path: t=exp(-g) (ACT) then ONE custom-DVE AFFINE_MUL_REDUCE
    den2 = (16t+16)*denom and r2 = 1/den2 (DVE), ATT = av*r2 (Pool).
    Removes the Pool affine + separate sigmoid reciprocal + one multiply.
  * rstd = exp(-0.5*ln(var+eps')): Ln+Exp live in the same ACT table
    (natural_log_exp_and_others) as the softmax Exp -> ONE table load total.
  * Residual rides the ff matmul as fp8 hi/lo DoubleRow (x16 = hi+lo, both
    slots through a stride-0 identity lhsT) instead of bf16 identity:
    halves residual PE cost, same precision (~0.4%).
  * LN normalize split ACT/Pool per 512-chunk to balance engines.
  * v-copies moved DVE -> Pool.
  * No zero-slot q/k tiles -> no big Pool memsets.

Scale ledger (fp8 ranges; e4m3 max finite = 240):
    Wq,Wk *8 ; Wv,Wg *64 ; x *1        -> q_ps=8q k_ps=8k v_ps=64v g_ps=64g
    exp arg = sc_ps * ES, ES=1/(64*sqrt(128)); bias via BT=(gamma*b+OFF)*SU,
        SU = 1/(ES*C), C=128, OFF=-3 (exp <= e^~2.5, fits fp8)
    sig path: t=exp(-g_ps/64)=e^-g (ACT); den2=(16t+16)*denom (DVE AMR);
        r2=1/den2 (DVE)
    ATT = av_ps * r2 = 4*attv*sig   (av_ps = 64*denom*attv)
    W_ff *4 -> ff_ps = 16*ff ; x16 = hi+lo fp8 ; h_ps = 16*(x+ff)
    eps' = 256*eps ; rstd = exp(-0.5*ln(var+eps'))
"""

import math

import numpy as np
import ml_dtypes

import concourse.bass as bass
import concourse.mybir as mybir
import concourse.tile as tile
from concourse import bacc
from concourse.bass_utils import run_bass_kernel_spmd

B, N, D, H, DH = 8, 1024, 1024, 8, 128
KT = D // 128
KTP = KT // 2
LN_EPS = 1e-5

F32 = mybir.dt.float32
BF16 = mybir.dt.bfloat16
FP8 = mybir.dt.float8e4
DR = mybir.MatmulPerfMode.DoubleRow
FP8NP = ml_dtypes.float8_e4m3
AF = mybir.ActivationFunctionType

SQ = 8.0        # q,k weight prescale
SV = 64.0       # v,g weight prescale
CID = 128.0     # identity-slot constant
ES = 1.0 / (SQ * SQ * math.sqrt(DH))     # exp() scale on scores psum
SU = 1.0 / (ES * CID)                    # bias prescale into BT
OFF = -3.0      # score offset (softmax-invariant), keeps exp in fp8 range
SA = 16.0       # h_ps = SA*(x+ff)
EPS2 = LN_EPS * SA * SA
NPAR = 4

_cache = {}


def _ident(nc, ap2d, fill):
    """diag(fill) into a zeroed [128,128] view."""
    nc.gpsimd.memset(ap2d, 0.0)
    nc.gpsimd.affine_select(
        out=ap2d,
        in_=ap2d,
        compare_op=mybir.AluOpType.not_equal,
        fill=fill,
        base=0,
        pattern=[[-1, 128]],
        channel_multiplier=1,
    )


def _build(flags):
    general_gamma, use_bff, use_lng, use_lnb = flags
    nc = bacc.Bacc("TRN2", target_bir_lowering=False)

    xt8_d = nc.dram_tensor("xt8", [128, KT, N], FP8, kind="ExternalInput")
    bt_shape = [H, 128, KT, N] if general_gamma else [128, KT, N]
    bt_d = nc.dram_tensor("bt", bt_shape, FP8, kind="ExternalInput")
    watt_d = nc.dram_tensor("watt", [H, 128, 4, KTP, 2, 128], FP8, kind="ExternalInput")
    wff_d = nc.dram_tensor("wff", [128, H, D], FP8, kind="ExternalInput")
    xhl_d = nc.dram_tensor("xhl", [KT, 128, 2, D], FP8, kind="ExternalInput")
    if use_bff:
        bff_d = nc.dram_tensor("bff", [1, D], F32, kind="ExternalInput")
    if use_lng:
        lng_d = nc.dram_tensor("lng", [1, D], F32, kind="ExternalInput")
    if use_lnb:
        lnb_d = nc.dram_tensor("lnb", [1, D], F32, kind="ExternalInput")
    out_d = nc.dram_tensor("out", [N, D], BF16, kind="ExternalOutput")

    with tile.TileContext(nc) as tc:
        with (
            tc.tile_pool(name="singles", bufs=1) as singles,
            tc.tile_pool(name="sb_w", bufs=3) as sb_w,
            tc.tile_pool(name="sb_sig", bufs=2) as sb_sig,
            tc.tile_pool(name="sb_v", bufs=3) as sb_v,
            tc.tile_pool(name="sb_dr", bufs=4) as sb_dr,
            tc.tile_pool(name="sb_st", bufs=4) as sb_st,
            tc.tile_pool(name="sb_o", bufs=3) as sb_o,
            tc.tile_pool(name="ps_sc", bufs=2, space="PSUM") as ps_sc,
            tc.tile_pool(name="ps_pr", bufs=2, space="PSUM") as ps_pr,
            tc.tile_pool(name="ps_tl", bufs=2, space="PSUM") as ps_tl,
        ):
            # ---- constants / resident tensors ----
            XT8 = singles.tile([128, KT, N], FP8, tag="XT8")
            wt0 = singles.tile([128, 4, KTP, 2, 128], FP8, tag="wt0")
            # head-0 critical path first: x^T half 0, then q/k/g weight
            # sections of head 0, then x^T half 1, then head-0 v weights.
            nc.sync.dma_start(out=XT8[:, :, 0:512], in_=xt8_d[:, :, 0:512])
            nc.sync.dma_start(out=wt0[:, 0:2, :, :, :], in_=watt_d[0, :, 0:2])
            nc.sync.dma_start(out=wt0[:, 3, :, :, :], in_=watt_d[0, :, 3])
            nc.sync.dma_start(out=XT8[:, :, 512:N], in_=xt8_d[:, :, 512:N])
            nc.sync.dma_start(out=wt0[:, 2, :, :, :], in_=watt_d[0, :, 2])

            QB = [singles.tile([128, 1 + KT, N], FP8, tag=f"QB{p}", name=f"QB{p}")
                  for p in range(NPAR)]
            KI = [singles.tile([128, KT + 1, 128], FP8, tag=f"KI{p}", name=f"KI{p}")
                  for p in range(NPAR)]
            PT2 = [singles.tile([128, KT, N], FP8, tag=f"PT{p}", name=f"PT{p}")
                   for p in range(NPAR)]
            if not general_gamma:
                # bias rows replicated into every QB buffer; QB0's first two
                # kt rows jump the queue (head 0's first scores), the rest
                # stagger into the head loop so watt/xr DMAs aren't starved.
                nc.sync.dma_start(out=QB[0][:, 1:3, :], in_=bt_d[:, 0:2, :])
                nc.sync.dma_start(out=QB[0][:, 3 : 1 + KT, :], in_=bt_d[:, 2:KT, :])
                nc.sync.dma_start(out=QB[1][:, 1 : 1 + KT, :], in_=bt_d[:, :, :])

            WFF8 = singles.tile([128, H, D], FP8, tag="WFF8")
            XR8 = singles.tile([128, KT, 2, D], FP8, tag="XR8")
            ATT = singles.tile([128, H, N], FP8, tag="ATT")
            ones2 = singles.tile([128, 2, 128], FP8, tag="ones2")
            nc.gpsimd.memset(ones2, 1.0)
            eps_t = singles.tile([128, 1], F32, tag="eps_t")
            nc.gpsimd.memset(eps_t, EPS2)
            ID1 = singles.tile([128, 128], FP8, tag="ID1")
            _ident(nc, ID1, 1.0)
            for p in range(NPAR):
                _ident(nc, KI[p][:, KT, :], CID)
            if use_bff:
                bffb = singles.tile([128, D], F32, tag="bffb")
                nc.sync.dma_start(
                    out=bffb,
                    in_=bass.AP(tensor=bff_d, offset=0, ap=[[0, 128], [1, D]]),
                )
            if use_lng:
                lngb = singles.tile([128, D], F32, tag="lngb")
                nc.sync.dma_start(
                    out=lngb,
                    in_=bass.AP(tensor=lng_d, offset=0, ap=[[0, 128], [1, D]]),
                )
            if use_lnb:
                lnbb = singles.tile([128, D], F32, tag="lnbb")
                nc.sync.dma_start(
                    out=lnbb,
                    in_=bass.AP(tensor=lnb_d, offset=0, ap=[[0, 128], [1, D]]),
                )

            # strided-AP helpers (slot-1 rides a different row of the tile)
            def ki_lhs(par, kt):
                t = KI[par][:, 0, :]
                return bass.AP(
                    tensor=t.tensor,
                    offset=kt * 128,
                    ap=[[(KT + 1) * 128, 128], [(KT - kt) * 128, 2], [1, 128]],
                )

            def qb_rhs(par, kt, c):
                t = QB[par][:, 0, :]
                return bass.AP(
                    tensor=t.tensor,
                    offset=c * 512,
                    ap=[[(1 + KT) * N, 128], [(1 + kt) * N, 2], [1, 512]],
                )

            def id_res():
                return bass.AP(
                    tensor=ID1[:, :].tensor,
                    offset=0,
                    ap=[[128, 128], [0, 2], [1, 128]],
                )

            # ---- per-head attention ----
            for h in range(H):
                par = h % NPAR
                QBp, KIp, PT = QB[par], KI[par], PT2[par]
                if h == 0:
                    wt = wt0
                else:
                    wt = sb_w.tile([128, 4, KTP, 2, 128], FP8, tag="wt", name="wt")
                    nc.sync.dma_start(out=wt, in_=watt_d[h])
                nc.sync.dma_start(out=XR8[:, h, :, :], in_=xhl_d[h])
                if h == 6:
                    nc.sync.dma_start(out=WFF8, in_=wff_d[:, :, :])
                if general_gamma:
                    nc.sync.dma_start(out=QBp[:, 1 : 1 + KT, :], in_=bt_d[h])
                elif h < 2:
                    nc.sync.dma_start(
                        out=QB[h + 2][:, 1 : 1 + KT, :], in_=bt_d[:, :, :]
                    )

                # - q -> QB row 0, k -> KI rows 0..7 (fp8 copies via DVE) -
                for j, dst in ((0, None), (1, None)):
                    for c in range(2):
                        pr = ps_pr.tile([128, 512], F32, tag="ps_pr", name=f"pr{j}{c}")
                        for ktp in range(KTP):
                            nc.tensor.matmul(
                                pr,
                                wt[:, j, ktp, :, :],
                                XT8[:, 2 * ktp : 2 * ktp + 2, c * 512 : (c + 1) * 512],
                                start=(ktp == 0),
                                stop=(ktp == KTP - 1),
                                perf_mode=DR,
                            )
                        if j == 0:
                            o_ap = QBp[:, 0, c * 512 : (c + 1) * 512]
                        else:
                            o_ap = KIp[:, 4 * c : 4 * c + 4, :]
                        nc.vector.tensor_copy(out=o_ap, in_=pr)

                # - gate projection into one scores-psum tile -> ONE
                #   t = exp(-g) over [128,1024] (ACT) -
                gp = ps_sc.tile([128, N], F32, tag="ps_sc", name="gp")
                sig_t = sb_sig.tile([128, N], BF16, tag="sig_t", name="sig_t")
                for c in range(2):
                    gr = gp[:, c * 512 : (c + 1) * 512]
                    for ktp in range(KTP):
                        nc.tensor.matmul(
                            gr,
                            wt[:, 3, ktp, :, :],
                            XT8[:, 2 * ktp : 2 * ktp + 2, c * 512 : (c + 1) * 512],
                            start=(ktp == 0),
                            stop=(ktp == KTP - 1),
                            perf_mode=DR,
                        )
                nc.scalar.activation(
                    out=sig_t,
                    in_=gp,
                    func=AF.Exp,
                    scale=-1.0 / SV,
                )
                # t2 = 16t+16 (Pool, sbuf); sig_r = 1/t2 = sigmoid(g)/16 (DVE)
                sig_r = sb_sig.tile([128, N], BF16, tag="sig_r", name="sig_r")
                nc.gpsimd.tensor_scalar(
                    sig_r, sig_t, SA, SA,
                    mybir.AluOpType.mult, mybir.AluOpType.add,
                )
                with nc.allow_low_precision(reason="gate in bf16; |err|~0.4% ok"):
                    nc.vector.reciprocal(sig_r, sig_r)

                # - v projection (natural [k-token, dh] layout) -
                v8 = sb_v.tile([128, KT, 128], FP8, tag="v8", name="v8")

                # - scores: one fused (k.q + bias) DR matmul per (kt, 512q);
                #   per-kt exp on ping-ponged [128,N] tiles; v interleaved -
                for kt in range(KT):
                    sc = ps_sc.tile([128, N], F32, tag="ps_sc", name=f"sc{kt}")
                    for c in range(2):
                        nc.tensor.matmul(
                            sc[:, c * 512 : (c + 1) * 512],
                            ki_lhs(par, kt),
                            qb_rhs(par, kt, c),
                            start=True,
                            stop=True,
                            perf_mode=DR,
                        )
                    nc.scalar.activation(
                        out=PT[:, kt, :],
                        in_=sc,
                        func=AF.Exp,
                        scale=ES,
                    )
                    if kt == 1 or kt == 3:
                        # v chunk while ACT chews on the exps
                        c = (kt - 1) // 2
                        vr = ps_tl.tile(
                            [128, 4, 128], F32, tag="ps_tl", name=f"vr{c}"
                        )
                        for nb4 in range(4):
                            nb = 4 * c + nb4
                            for ktp in range(KTP):
                                nc.tensor.matmul(
                                    vr[:, nb4, :],
                                    XT8[:, 2 * ktp : 2 * ktp + 2, nb * 128 : (nb + 1) * 128],
                                    wt[:, 2, ktp, :, :],
                                    start=(ktp == 0),
                                    stop=(ktp == KTP - 1),
                                    perf_mode=DR,
                                )
                        nc.vector.tensor_copy(
                            out=v8[:, 4 * c : 4 * c + 4, :], in_=vr
                        )

                # - denominators -> rbb = 1/denom (DVE, pre-broadcast) -
                rbb = sb_dr.tile([128, N], F32, tag="rbb", name="rbb")
                for c in range(2):
                    dn = ps_tl.tile([128, 512], F32, tag="ps_tl", name=f"dn{c}")
                    for ktp in range(KTP):
                        nc.tensor.matmul(
                            dn,
                            ones2,
                            PT[:, 2 * ktp : 2 * ktp + 2, c * 512 : (c + 1) * 512],
                            start=(ktp == 0),
                            stop=(ktp == KTP - 1),
                            perf_mode=DR,
                        )
                    nc.vector.reciprocal(
                        rbb[:, c * 512 : (c + 1) * 512], dn
                    )

                # - av = v8^T @ PT ; t1 = av / t2 (DVE divide);
                #   ATT = t1 * rbb (Pool, sbuf-only) -
                for c in range(2):
                    av = ps_tl.tile([128, 512], F32, tag="ps_tl", name=f"av{c}")
                    for ktp in range(KTP):
                        nc.tensor.matmul(
                            av,
                            v8[:, 2 * ktp : 2 * ktp + 2, :],
                            PT[:, 2 * ktp : 2 * ktp + 2, c * 512 : (c + 1) * 512],
                            start=(ktp == 0),
                            stop=(ktp == KTP - 1),
                            perf_mode=DR,
                        )
                    t1 = sb_dr.tile([128, 512], BF16, tag="t1", name=f"t1{c}")
                    nc.vector.tensor_mul(
                        t1, av, sig_r[:, c * 512 : (c + 1) * 512]
                    )
                    nc.gpsimd.tensor_mul(
                        ATT[:, h, c * 512 : (c + 1) * 512],
                        t1,
                        rbb[:, c * 512 : (c + 1) * 512],
                    )

            # ---- output projection + residual + LayerNorm ----
            for nt in range(N // 128):
                ffs = []
                stats = sb_st.tile([128, 2, 6], F32, tag="stats", name="stats")
                r4 = nt % 4
                if r4 >= 2:
                    ffsc = ps_sc.tile([128, N], F32, tag="ps_sc", name="ffp")
                for c in range(2):
                    if r4 == 0:
                        ff = ps_pr.tile([128, 512], F32, tag="ps_pr", name=f"ff{c}")
                    elif r4 == 1:
                        ff = ps_tl.tile([128, 512], F32, tag="ps_tl", name=f"ff{c}")
                    else:
                        ff = ffsc[:, c * 512 : (c + 1) * 512]
                    ffs.append(ff)
                    # residual: ff = I^T@xhi + I^T@xlo (fp8 hi/lo DoubleRow)
                    nc.tensor.matmul(
                        ff,
                        id_res(),
                        XR8[:, nt, :, c * 512 : (c + 1) * 512],
                        start=True,
                        stop=False,
                        perf_mode=DR,
                    )
                    for fp4 in range(KTP):
                        nc.tensor.matmul(
                            ff,
                            ATT[:, 2 * fp4 : 2 * fp4 + 2, nt * 128 : (nt + 1) * 128],
                            WFF8[:, 2 * fp4 : 2 * fp4 + 2, c * 512 : (c + 1) * 512],
                            start=False,
                            stop=(fp4 == KTP - 1),
                            perf_mode=DR,
                        )
                    if use_bff:
                        nc.vector.tensor_add(
                            ff, ff, bffb[:, c * 512 : (c + 1) * 512]
                        )
                    nc.vector.bn_stats(out=stats[:, c, :], in_=ff)
                mv = sb_st.tile([128, 2], F32, tag="mv", name="mv")
                nc.vector.bn_aggr(out=mv, in_=stats)
                rstd = sb_st.tile([128, 1], F32, tag="rstd", name="rstd")
                nc.scalar.activation(
                    out=rstd,
                    in_=mv[:, 1:2],
                    func=AF.Abs_reciprocal_sqrt,
                    bias=eps_t,
                    scale=1.0,
                )
                mb = sb_st.tile([128, 1], F32, tag="mb", name="mb")
                nc.vector.tensor_scalar(
                    mb, mv[:, 0:1], -1.0, rstd,
                    mybir.AluOpType.mult, mybir.AluOpType.mult,
                )
                o = sb_o.tile([128, D], BF16, tag="o", name="o")
                for c in range(2):
                    nc.scalar.activation(
                        out=o[:, c * 512 : (c + 1) * 512],
                        in_=ffs[c],
                        func=AF.Identity,
                        bias=mb,
                        scale=rstd,
                    )
                    if use_lng:
                        nc.vector.tensor_mul(
                            o[:, c * 512 : (c + 1) * 512],
                            o[:, c * 512 : (c + 1) * 512],
                            lngb[:, c * 512 : (c + 1) * 512],
                        )
                    if use_lnb:
                        nc.vector.tensor_add(
                            o[:, c * 512 : (c + 1) * 512],
                            o[:, c * 512 : (c + 1) * 512],
                            lnbb[:, c * 512 : (c + 1) * 512],
                        )
                    nc.sync.dma_start(
                        out=out_d[
                            nt * 128 : (nt + 1) * 128, c * 512 : (c + 1) * 512
                        ],
                        in_=o[:, c * 512 : (c + 1) * 512],
                    )

    nc.finalize()
    return nc


def get_nc(flags=(False, False, False, False)):
    if flags not in _cache:
        _cache[flags] = _build(flags)
    return _cache[flags]


def _fp8(a):
    return np.asarray(a, dtype=np.float32).astype(FP8NP)


def kernel(x, mask, bias, gamma_f, W_att, W_ff, b_ff, ln_g, ln_b):
    x = np.asarray(x, dtype=np.float32)
    mask = np.asarray(mask)
    bias = np.asarray(bias, dtype=np.float32)
    gamma_f = np.asarray(gamma_f, dtype=np.float32)
    W_att = np.asarray(W_att, dtype=np.float32)
    W_ff = np.asarray(W_ff, dtype=np.float32)
    b_ff = np.asarray(b_ff, dtype=np.float32)
    ln_g = np.asarray(ln_g, dtype=np.float32)
    ln_b = np.asarray(ln_b, dtype=np.float32)

    general_gamma = not np.all(gamma_f == 1.0)
    use_bff = bool(np.any(b_ff != 0.0))
    use_lng = not np.all(ln_g == 1.0)
    use_lnb = bool(np.any(ln_b != 0.0))
    flags = (general_gamma, use_bff, use_lng, use_lnb)
    nc = get_nc(flags)

    # watt8[h, p, j, ktp, i, fcol] = W_att[ktp*256 + i*128 + p, sect_j + h*128
    #   + fcol] * scale_j   (j: 0=q 1=k 2=v 3=g)
    w4 = W_att.reshape(KTP, 2, 128, 4, H, DH)  # [ktp, i, p, sect, h, fcol]
    watt8 = np.empty((H, 128, 4, KTP, 2, 128), dtype=FP8NP)
    scales = (SQ, SQ, SV, SV)
    for j in range(4):
        # -> [h, p, ktp, i, fcol]
        sect = np.transpose(w4[:, :, :, j, :, :], (3, 2, 0, 1, 4))
        watt8[:, :, j, :, :, :] = _fp8(sect * scales[j])

    wff8 = _fp8(4.0 * W_ff.reshape(H, 128, D).transpose(1, 0, 2))

    valid = ~mask[:, 0, :, :]  # [B, N, N] True where kept

    in_maps = []
    for b in range(B):
        # x^T tiled: XT8[p, kt, n] = x[n, kt*128 + p]
        xt8 = _fp8(np.ascontiguousarray(
            x[b].T.reshape(KT, 128, N).transpose(1, 0, 2)
        ))
        # BT[p, kt, q] = (gamma*bias[q, kt*128+p] + OFF)*SU, masked -> -240
        btr = np.ascontiguousarray(bias[b].T)  # [k, q]
        btr = btr.reshape(KT, 128, N).transpose(1, 0, 2)  # [p, kt, q]
        vtr = np.ascontiguousarray(valid[b].T).reshape(KT, 128, N).transpose(1, 0, 2)
        if general_gamma:
            bt8 = np.empty((H, 128, KT, N), dtype=FP8NP)
            for h in range(H):
                uh = np.clip((gamma_f[h] * btr + OFF) * SU, -239.0, 239.0)
                bt8[h] = np.where(vtr, uh, np.float32(-240.0)).astype(FP8NP)
        else:
            uh = np.clip((btr + OFF) * SU, -239.0, 239.0)
            bt8 = np.where(vtr, uh, np.float32(-240.0)).astype(FP8NP)
        # residual hi/lo: x16 = hi + lo in fp8
        x16 = SA * x[b]
        hi = x16.astype(FP8NP)
        lo = (x16 - hi.astype(np.float32)).astype(FP8NP)
        xhl = np.stack(
            [hi.reshape(KT, 128, D), lo.reshape(KT, 128, D)], axis=2
        )  # [KT, 128, 2, D]
        im = {
            "xt8": xt8,
            "bt": bt8,
            "watt": watt8,
            "wff": wff8,
            "xhl": np.ascontiguousarray(xhl),
        }
        if use_bff:
            im["bff"] = SA * b_ff.reshape(1, D)
        if use_lng:
            im["lng"] = ln_g.reshape(1, D)
        if use_lnb:
            im["lnb"] = ln_b.reshape(1, D)
        in_maps.append(im)

    res = run_bass_kernel_spmd(nc, in_maps, core_ids=list(range(B)))
    out = np.stack([res.results[b]["out"] for b in range(B)], axis=0)
    return out.astype(np.float32)
